# revision 1
# baseline (speedup 1.0000x reference)
import time
from contextlib import ExitStack

import numpy as np

BOS_IDX = 0
NCORES = 8
L = 128
M_SLOTS = 64

LAST = {}


def _perron_c(transitions):
    W64 = np.exp(transitions.astype(np.float64))
    v = np.ones(L)
    for _ in range(200):
        v = W64 @ v
        v /= np.linalg.norm(v)
    lam1 = float(v @ W64 @ v) / float(v @ v)
    return float(np.log(lam1) + 0.5)


def _host_prep(logits, transitions, lens, M=M_SLOTS):
    B, S, Lc = logits.shape
    assert Lc == L
    lens = np.asarray(lens).astype(np.int64)

    c = _perron_c(transitions)
    Wp64 = np.exp(transitions.astype(np.float64) - c)
    wf = np.ascontiguousarray(Wp64.T).astype(np.float16)
    wb = np.ascontiguousarray(Wp64).astype(np.float16)
    r_pad = (1.0 / (Wp64 @ np.ones(L))).astype(np.float16)
    z_pad = np.linalg.solve(Wp64.T, np.ones(L)).astype(np.float16)

    elog = np.exp(logits.astype(np.float32)).astype(np.float16)

    fwd_jobs = []
    bwd_jobs = []
    seq_k = np.zeros(B, np.int64)
    for b in range(B):
        l = int(lens[b])
        k = max(1, -(-l // M))
        seq_k[b] = k
        cuts = [min(i * M, l) for i in range(k + 1)]
        for j in range(k - 1):
            fwd_jobs.append((cuts[j + 1] - cuts[j], b, cuts[j], cuts[j + 1]))
            bwd_jobs.append((cuts[j + 1] - cuts[j], b, cuts[j], cuts[j + 1]))
        bwd_jobs.append((cuts[k] - cuts[k - 1], b, cuts[k - 1], cuts[k]))

    def deal(jobs):
        jobs = sorted(jobs, key=lambda j: -j[0])
        while len(jobs) % NCORES:
            jobs.append((0, -1, 0, 0))
        nrank = len(jobs) // NCORES
        ranks = [[jobs[i * NCORES + m] for i in range(nrank)]
                 for m in range(NCORES)]
        RL = np.array([max(ranks[m][i][0] for m in range(NCORES))
                       for i in range(nrank)], np.int64)
        return ranks, RL

    ranks_f, FL = deal(fwd_jobs)
    ranks_b, BL = deal(bwd_jobs)
    COLS = max(len(FL), len(BL))
    NSLOT = int(max(FL.max() if len(FL) else 0,
                    BL.max() if len(BL) else 0))
    assert NSLOT <= M

    place = {}
    for m in range(NCORES):
        for i, (ln_, b, t0, t1) in enumerate(ranks_f[m]):
            if ln_ > 0:
                place[(b, 'f', t0)] = (m, i)
        for i, (ln_, b, t0, t1) in enumerate(ranks_b[m]):
            if ln_ > 0:
                place[(b, 'b', t0)] = (m, i)

    efs, ebs = [], []
    for m in range(NCORES):
        ef = np.zeros((NSLOT, L, COLS), np.float16)
        eb = np.zeros((NSLOT, L, COLS), np.float16)
        for i, (ln_, b, t0, t1) in enumerate(ranks_f[m]):
            if ln_ == 0:
                continue
            pad = int(FL[i]) - ln_
            if pad:
                ef[:pad, :, i] = r_pad[None, :]
            ef[pad:FL[i], :, i] = elog[b, t0:t1, :]
        for i, (ln_, b, t0, t1) in enumerate(ranks_b[m]):
            if ln_ == 0:
                continue
            pad = int(BL[i]) - ln_
            if pad:
                eb[:pad, :, i] = z_pad[None, :]
            eb[pad:BL[i], :, i] = elog[b, t0:t1, :][::-1]
        efs.append(np.ascontiguousarray(
            ef.transpose(1, 0, 2).reshape(L, NSLOT * COLS)))
        ebs.append(np.ascontiguousarray(
            eb.transpose(1, 0, 2).reshape(L, NSLOT * COLS)))

    def runs(ks):
        out = []
        for kk in sorted(ks):
            if out and out[-1][1] == kk:
                out[-1] = (out[-1][0], kk + 1)
            else:
                out.append((kk, kk + 1))
        return out

    fwd_caps, bwd_caps = {}, {}
    for i in range(len(FL)):
        if FL[i] >= 1:
            fwd_caps.setdefault(int(FL[i] - 1), []).append(i)
    for i in range(len(BL)):
        if BL[i] >= 1:
            bwd_caps.setdefault(int(BL[i] - 1), []).append(i)
    fwd_caps = {s: runs(v) for s, v in fwd_caps.items()}
    bwd_caps = {s: runs(v) for s, v in bwd_caps.items()}

    nf = np.zeros(NSLOT, np.int64)
    nb = np.zeros(NSLOT, np.int64)
    for s in range(NSLOT):
        af = np.nonzero(FL > s)[0]
        ab = np.nonzero(BL > s)[0]
        nf[s] = (af.max() + 1) if af.size else 0
        nb[s] = (ab.max() + 1) if ab.size else 0

    return dict(c=c, wf=wf, wb=wb, efs=efs, ebs=ebs, NSLOT=NSLOT,
                COLS=COLS, fwd_caps=fwd_caps, bwd_caps=bwd_caps,
                nf=nf, nb=nb, place=place, lens=lens, seq_k=seq_k, M=M)


def _host_combine(prep, capf, capb):
    c = prep["c"]
    lens = prep["lens"]
    seq_k = prep["seq_k"]
    place = prep["place"]
    M = prep["M"]
    B = len(lens)
    ones = np.ones(L, np.float64)
    e_bos = np.zeros(L, np.float64)
    e_bos[BOS_IDX] = 1.0

    def vec(kind, b, t0):
        m, i = place[(b, kind, t0)]
        t = capf[m][:, i] if kind == 'f' else capb[m][:, i]
        return t.astype(np.float64)

    logZ = np.empty(B, np.float64)
    for b in range(B):
        l = int(lens[b])
        k = int(seq_k[b])
        cuts = [min(i * M, l) for i in range(k + 1)]
        H = vec('b', b, cuts[k - 1])
        if k == 1:
            logZ[b] = np.log(H[BOS_IDX]) + c * l
            continue
        A_last = vec('f', b, cuts[k - 2])
        lz = np.log(H @ A_last)
        for j in range(2, k):
            Bj = vec('b', b, cuts[j - 1])
            Aprev = vec('f', b, cuts[j - 2])
            lz += np.log(Bj @ Aprev) - np.log(Bj @ ones)
        B1 = vec('b', b, cuts[0])
        lz += np.log(B1[BOS_IDX]) - np.log(B1 @ ones)
        logZ[b] = lz + c * l
    return logZ.astype(np.float32)


def _build_bass(NSLOT, COLS, fwd_caps, bwd_caps, chunk_slots=8, repeat=1,
                probe_same_weights=False, nf=None, nb=None, timing=False,
                caps=True, fwd_only=False):
    import concourse.bacc as bacc
    import concourse.mybir as mybir
    import concourse.tile as tile

    f32 = mybir.dt.float32
    f16 = mybir.dt.float16
    nc = bacc.Bacc("TRN2", target_bir_lowering=False, debug=False,
                   num_devices=NCORES)

    stream_kind = "Internal" if timing else "ExternalInput"
    ef_d = nc.dram_tensor("ef", [L, NSLOT * COLS], f16, kind=stream_kind).ap()
    eb_d = nc.dram_tensor("eb", [L, NSLOT * COLS], f16, kind=stream_kind).ap()
    wf_d = nc.dram_tensor("wf", [L, L], f16, kind="ExternalInput").ap()
    wb_d = nc.dram_tensor("wb", [L, L], f16, kind="ExternalInput").ap()
    capf_d = nc.dram_tensor("capf", [L, COLS], f32, kind="ExternalOutput").ap()
    capb_d = nc.dram_tensor("capb", [L, COLS], f32, kind="ExternalOutput").ap()

    with tile.TileContext(nc) as tc, ExitStack() as ctx:
        cpool = ctx.enter_context(tc.tile_pool(name="const", bufs=1))
        spool = ctx.enter_context(tc.tile_pool(name="state", bufs=3))
        strm = ctx.enter_context(tc.tile_pool(name="stream", bufs=3))
        pspool = ctx.enter_context(tc.tile_pool(name="ps", bufs=2, space="PSUM"))

        wf_t = cpool.tile([L, L], f16, tag="wf")
        nc.sync.dma_start(wf_t[:], wf_d[:])
        wb_t = cpool.tile([L, L], f16, tag="wb")
        nc.sync.dma_start(wb_t[:], wb_d[:])

        capF = cpool.tile([L, COLS], f32, tag="capF")
        nc.vector.memset(capF[:], 1.0)
        capB = cpool.tile([L, COLS], f32, tag="capB")
        nc.vector.memset(capB[:], 1.0)

        p0 = spool.tile([L, COLS], f16, tag="p")
        nc.vector.memset(p0[:], 1.0)

        bounds = [0]
        while bounds[-1] < NSLOT:
            step = 4 if bounds[-1] == 0 else chunk_slots
            bounds.append(min(NSLOT, bounds[-1] + step))
        chunks = list(zip(bounds[:-1], bounds[1:]))

        prev_p = None
        prev_hb = None
        for rep in range(repeat):
            if rep == 0:
                p = p0
            else:
                p = spool.tile([L, COLS], f16, tag="p")
                nc.vector.tensor_copy(p[:], p0[:])
                nc.scalar.copy(p[:, :prev_p.shape[1]], prev_p[:])
            hb = None
            for s0, s1 in chunks:
                ef_sb = strm.tile([L, (s1 - s0) * COLS], f16, tag="ef")
                nc.sync.dma_start(ef_sb[:], ef_d[:, s0 * COLS:s1 * COLS])
                eb_sb = strm.tile([L, (s1 - s0) * COLS], f16, tag="eb")
                nc.sync.dma_start(eb_sb[:], eb_d[:, s0 * COLS:s1 * COLS])
                for s in range(s0, s1):
                    j = s - s0
                    wf_n = COLS if nf is None else int(nf[s])
                    wb_n = COLS if nb is None else int(nb[s])
                    if wf_n > 0:
                        efsl = ef_sb[:, j * COLS:j * COLS + wf_n]
                        qf = pspool.tile([L, wf_n], f32, tag="qf")
                        nc.tensor.matmul(qf[:], wf_t[:], p[:, :wf_n])
                        p = spool.tile([L, wf_n], f16, tag="p")
                        nc.vector.tensor_mul(p[:], qf[:], efsl)
                        if caps and rep == 0:
                            for lo, hi in fwd_caps.get(s, []):
                                nc.scalar.copy(capF[:, lo:hi], p[:, lo:hi])
                    if wb_n > 0 and not fwd_only:
                        ebsl = eb_sb[:, j * COLS:j * COLS + wb_n]
                        if hb is None and rep > 0:
                            vb = spool.tile([L, wb_n], f16, tag="vb")
                            nc.vector.tensor_copy(vb[:], ebsl)
                            pw = min(prev_hb.shape[1], wb_n)
                            nc.vector.tensor_mul(
                                vb[:, :pw], prev_hb[:, :pw], ebsl[:, :pw])
                            vb_ap = vb[:]
                        elif hb is None:
                            vb_ap = ebsl
                        else:
                            vb = spool.tile([L, wb_n], f16, tag="vb")
                            nc.vector.tensor_mul(vb[:], hb[:, :wb_n], ebsl)
                            vb_ap = vb[:]
                        hb = pspool.tile([L, wb_n], f32, tag="hb")
                        nc.tensor.matmul(
                            hb[:], (wf_t if probe_same_weights else wb_t)[:],
                            vb_ap)
                        if caps and rep == 0:
                            for lo, hi in bwd_caps.get(s, []):
                                nc.scalar.copy(capB[:, lo:hi], hb[:, lo:hi])
            prev_p, prev_hb = p, hb

        nc.sync.dma_start(capf_d[:], capF[:])
        nc.sync.dma_start(capb_d[:], capB[:])

    nc.compile()
    return nc


def kernel(logits, transitions, lens):
    from concourse.bass_utils import run_bass_kernel_spmd

    logits = np.asarray(logits, dtype=np.float32)
    transitions = np.asarray(transitions, dtype=np.float32)
    lens_in = np.asarray(lens)

    prep = _host_prep(logits, transitions, lens_in, M=M_SLOTS)

    t0 = time.time()
    nc = _build_bass(prep["NSLOT"], prep["COLS"], prep["fwd_caps"],
                     prep["bwd_caps"], nf=prep["nf"], nb=prep["nb"])
    t1 = time.time()

    in_maps = [{"ef": prep["efs"][m], "eb": prep["ebs"][m],
                "wf": prep["wf"], "wb": prep["wb"]}
               for m in range(NCORES)]
    try:
        r = run_bass_kernel_spmd(nc, in_maps, core_ids=list(range(NCORES)))
    except Exception:
        time.sleep(10)
        r = run_bass_kernel_spmd(nc, in_maps, core_ids=list(range(NCORES)))
    t2 = time.time()

    capf = [r.results[m]["capf"] for m in range(NCORES)]
    capb = [r.results[m]["capb"] for m in range(NCORES)]
    out = _host_combine(prep, capf, capb)

    LAST.clear()
    LAST.update(build_s=t1 - t0, run_s=t2 - t1, results=r,
                exec_time_ns=r.exec_time_ns, nslot=prep["NSLOT"],
                cols=prep["COLS"])
    return out


if __name__ == "__main__":
    rng = np.random.default_rng(0)
    B, S = 512, 512
    logits = rng.standard_normal((B, S, L), dtype=np.float32)
    lens = rng.integers(1, S + 1, size=B).astype(np.int64)
    transitions = rng.standard_normal((L, L)).astype(np.float32)
    out = kernel(logits=logits, transitions=transitions, lens=lens)
    print("out[:8] =", out[:8])
    print("timings:", {k: LAST[k] for k in ("build_s", "run_s", "nslot")})



# revision 4
# speedup vs baseline: 244.5807x; 244.5807x over previous
import time
from contextlib import ExitStack

import numpy as np

BOS_IDX = 0
NCORES = 8
L = 128
M_SLOTS = 64

LAST = {}


def _perron_c(transitions):
    W64 = np.exp(transitions.astype(np.float64))
    v = np.ones(L)
    for _ in range(200):
        v = W64 @ v
        v /= np.linalg.norm(v)
    lam1 = float(v @ W64 @ v) / float(v @ v)
    return float(np.log(lam1) + 0.5)


def _host_prep(logits, transitions, lens, M=M_SLOTS):
    B, S, Lc = logits.shape
    assert Lc == L
    lens = np.asarray(lens).astype(np.int64)

    c = _perron_c(transitions)
    Wp64 = np.exp(transitions.astype(np.float64) - c)
    wf = np.ascontiguousarray(Wp64.T).astype(np.float16)
    wb = np.ascontiguousarray(Wp64).astype(np.float16)
    r_pad = (1.0 / (Wp64 @ np.ones(L))).astype(np.float16)
    z_pad = np.linalg.solve(Wp64.T, np.ones(L)).astype(np.float16)

    elog = np.exp(logits.astype(np.float32)).astype(np.float16)

    fwd_jobs = []
    bwd_jobs = []
    seq_k = np.zeros(B, np.int64)
    for b in range(B):
        l = int(lens[b])
        k = max(1, -(-l // M))
        seq_k[b] = k
        cuts = [min(i * M, l) for i in range(k + 1)]
        for j in range(k - 1):
            fwd_jobs.append((cuts[j + 1] - cuts[j], b, cuts[j], cuts[j + 1]))
            bwd_jobs.append((cuts[j + 1] - cuts[j], b, cuts[j], cuts[j + 1]))
        bwd_jobs.append((cuts[k] - cuts[k - 1], b, cuts[k - 1], cuts[k]))

    def deal(jobs):
        jobs = sorted(jobs, key=lambda j: -j[0])
        while len(jobs) % NCORES:
            jobs.append((0, -1, 0, 0))
        nrank = len(jobs) // NCORES
        ranks = [[jobs[i * NCORES + m] for i in range(nrank)]
                 for m in range(NCORES)]
        RL = np.array([max(ranks[m][i][0] for m in range(NCORES))
                       for i in range(nrank)], np.int64)
        return ranks, RL

    ranks_f, FL = deal(fwd_jobs)
    ranks_b, BL = deal(bwd_jobs)
    COLS = max(len(FL), len(BL))
    NSLOT = int(max(FL.max() if len(FL) else 0,
                    BL.max() if len(BL) else 0))
    assert NSLOT <= M

    place = {}
    for m in range(NCORES):
        for i, (ln_, b, t0, t1) in enumerate(ranks_f[m]):
            if ln_ > 0:
                place[(b, 'f', t0)] = (m, i)
        for i, (ln_, b, t0, t1) in enumerate(ranks_b[m]):
            if ln_ > 0:
                place[(b, 'b', t0)] = (m, i)

    efs, ebs = [], []
    for m in range(NCORES):
        ef = np.zeros((NSLOT, L, COLS), np.float16)
        eb = np.zeros((NSLOT, L, COLS), np.float16)
        for i, (ln_, b, t0, t1) in enumerate(ranks_f[m]):
            if ln_ == 0:
                continue
            pad = int(FL[i]) - ln_
            if pad:
                ef[:pad, :, i] = r_pad[None, :]
            ef[pad:FL[i], :, i] = elog[b, t0:t1, :]
        for i, (ln_, b, t0, t1) in enumerate(ranks_b[m]):
            if ln_ == 0:
                continue
            pad = int(BL[i]) - ln_
            if pad:
                eb[:pad, :, i] = z_pad[None, :]
            eb[pad:BL[i], :, i] = elog[b, t0:t1, :][::-1]
        efs.append(np.ascontiguousarray(
            ef.transpose(1, 0, 2).reshape(L, NSLOT * COLS)))
        ebs.append(np.ascontiguousarray(
            eb.transpose(1, 0, 2).reshape(L, NSLOT * COLS)))

    def runs(ks):
        out = []
        for kk in sorted(ks):
            if out and out[-1][1] == kk:
                out[-1] = (out[-1][0], kk + 1)
            else:
                out.append((kk, kk + 1))
        return out

    fwd_caps, bwd_caps = {}, {}
    for i in range(len(FL)):
        if FL[i] >= 1:
            fwd_caps.setdefault(int(FL[i] - 1), []).append(i)
    for i in range(len(BL)):
        if BL[i] >= 1:
            bwd_caps.setdefault(int(BL[i] - 1), []).append(i)
    fwd_caps = {s: runs(v) for s, v in fwd_caps.items()}
    bwd_caps = {s: runs(v) for s, v in bwd_caps.items()}

    nf = np.zeros(NSLOT, np.int64)
    nb = np.zeros(NSLOT, np.int64)
    for s in range(NSLOT):
        af = np.nonzero(FL > s)[0]
        ab = np.nonzero(BL > s)[0]
        nf[s] = (af.max() + 1) if af.size else 0
        nb[s] = (ab.max() + 1) if ab.size else 0

    return dict(c=c, wf=wf, wb=wb, efs=efs, ebs=ebs, NSLOT=NSLOT,
                COLS=COLS, fwd_caps=fwd_caps, bwd_caps=bwd_caps,
                nf=nf, nb=nb, place=place, lens=lens, seq_k=seq_k, M=M)


def _host_combine(prep, capf, capb):
    c = prep["c"]
    lens = prep["lens"]
    seq_k = prep["seq_k"]
    place = prep["place"]
    M = prep["M"]
    B = len(lens)
    ones = np.ones(L, np.float64)
    e_bos = np.zeros(L, np.float64)
    e_bos[BOS_IDX] = 1.0

    def vec(kind, b, t0):
        m, i = place[(b, kind, t0)]
        t = capf[m][:, i] if kind == 'f' else capb[m][:, i]
        return t.astype(np.float64)

    logZ = np.empty(B, np.float64)
    for b in range(B):
        l = int(lens[b])
        k = int(seq_k[b])
        cuts = [min(i * M, l) for i in range(k + 1)]
        H = vec('b', b, cuts[k - 1])
        if k == 1:
            logZ[b] = np.log(H[BOS_IDX]) + c * l
            continue
        A_last = vec('f', b, cuts[k - 2])
        lz = np.log(H @ A_last)
        for j in range(2, k):
            Bj = vec('b', b, cuts[j - 1])
            Aprev = vec('f', b, cuts[j - 2])
            lz += np.log(Bj @ Aprev) - np.log(Bj @ ones)
        B1 = vec('b', b, cuts[0])
        lz += np.log(B1[BOS_IDX]) - np.log(B1 @ ones)
        logZ[b] = lz + c * l
    return logZ.astype(np.float32)


def _build_bass(NSLOT, COLS, fwd_caps, bwd_caps, chunk_slots=8, repeat=1,
                probe_same_weights=False, nf=None, nb=None, timing=False,
                caps=True, fwd_only=False):
    import concourse.bacc as bacc
    import concourse.mybir as mybir
    import concourse.tile as tile

    f32 = mybir.dt.float32
    f16 = mybir.dt.float16
    nc = bacc.Bacc("TRN2", target_bir_lowering=False, debug=False,
                   num_devices=NCORES)

    stream_kind = "Internal" if timing else "ExternalInput"
    ef_d = nc.dram_tensor("ef", [L, NSLOT * COLS], f16, kind=stream_kind).ap()
    eb_d = nc.dram_tensor("eb", [L, NSLOT * COLS], f16, kind=stream_kind).ap()
    wf_d = nc.dram_tensor("wf", [L, L], f16, kind="ExternalInput").ap()
    wb_d = nc.dram_tensor("wb", [L, L], f16, kind="ExternalInput").ap()
    capf_d = nc.dram_tensor("capf", [L, COLS], f32, kind="ExternalOutput").ap()
    capb_d = nc.dram_tensor("capb", [L, COLS], f32, kind="ExternalOutput").ap()

    with tile.TileContext(nc) as tc, ExitStack() as ctx:
        cpool = ctx.enter_context(tc.tile_pool(name="const", bufs=1))
        spool = ctx.enter_context(tc.tile_pool(name="state", bufs=3))
        strm = ctx.enter_context(tc.tile_pool(name="stream", bufs=3))
        pspool = ctx.enter_context(tc.tile_pool(name="ps", bufs=2, space="PSUM"))

        wf_t = cpool.tile([L, L], f16, tag="wf")
        nc.sync.dma_start(wf_t[:], wf_d[:])
        wb_t = cpool.tile([L, L], f16, tag="wb")
        nc.sync.dma_start(wb_t[:], wb_d[:])

        capF = cpool.tile([L, COLS], f32, tag="capF")
        nc.vector.memset(capF[:], 1.0)
        capB = cpool.tile([L, COLS], f32, tag="capB")
        nc.vector.memset(capB[:], 1.0)

        serial = timing
        if serial:
            pcar = cpool.tile([L, COLS], f16, tag="pcar")
            nc.vector.memset(pcar[:], 1.0)
            hcar = cpool.tile([L, COLS], f16, tag="hcar")
            nc.vector.memset(hcar[:], 1.0)

        bounds = [0]
        while bounds[-1] < NSLOT:
            step = 4 if bounds[-1] == 0 else chunk_slots
            bounds.append(min(NSLOT, bounds[-1] + step))
        chunks = list(zip(bounds[:-1], bounds[1:]))

        def one_pass():
            p = spool.tile([L, COLS], f16, tag="p")
            nc.vector.memset(p[:], 1.0)
            if serial:
                nc.scalar.copy(p[:, :8], pcar[:, :8])
            hb = None
            for s0, s1 in chunks:
                ef_sb = strm.tile([L, (s1 - s0) * COLS], f16, tag="ef")
                nc.sync.dma_start(ef_sb[:], ef_d[:, s0 * COLS:s1 * COLS])
                eb_sb = strm.tile([L, (s1 - s0) * COLS], f16, tag="eb")
                nc.sync.dma_start(eb_sb[:], eb_d[:, s0 * COLS:s1 * COLS])
                for s in range(s0, s1):
                    j = s - s0
                    wf_n = COLS if nf is None else int(nf[s])
                    wb_n = COLS if nb is None else int(nb[s])
                    if wf_n > 0:
                        efsl = ef_sb[:, j * COLS:j * COLS + wf_n]
                        qf = pspool.tile([L, wf_n], f32, tag="qf")
                        nc.tensor.matmul(qf[:], wf_t[:], p[:, :wf_n])
                        p = spool.tile([L, wf_n], f16, tag="p")
                        nc.vector.tensor_mul(p[:], qf[:], efsl)
                        if caps:
                            for lo, hi in fwd_caps.get(s, []):
                                nc.scalar.copy(capF[:, lo:hi], p[:, lo:hi])
                    if wb_n > 0 and not fwd_only:
                        ebsl = eb_sb[:, j * COLS:j * COLS + wb_n]
                        if hb is None and serial:
                            vb = spool.tile([L, wb_n], f16, tag="vb")
                            nc.vector.tensor_mul(
                                vb[:], hcar[:, :wb_n], ebsl)
                            vb_ap = vb[:]
                        elif hb is None:
                            vb_ap = ebsl
                        else:
                            vb = spool.tile([L, wb_n], f16, tag="vb")
                            nc.vector.tensor_mul(vb[:], hb[:, :wb_n], ebsl)
                            vb_ap = vb[:]
                        hb = pspool.tile([L, wb_n], f32, tag="hb")
                        nc.tensor.matmul(
                            hb[:], (wf_t if probe_same_weights else wb_t)[:],
                            vb_ap)
                        if caps:
                            for lo, hi in bwd_caps.get(s, []):
                                nc.scalar.copy(capB[:, lo:hi], hb[:, lo:hi])
            if serial:
                nc.scalar.copy(pcar[:, :p.shape[1]], p[:])
                if hb is not None:
                    nc.scalar.copy(hcar[:, :hb.shape[1]], hb[:])

        if serial:
            with tc.For_i(0, repeat):
                one_pass()
        else:
            assert repeat == 1
            one_pass()

        nc.sync.dma_start(capf_d[:], capF[:])
        nc.sync.dma_start(capb_d[:], capB[:])

    nc.compile()
    return nc


def kernel(logits, transitions, lens):
    from concourse.bass_utils import run_bass_kernel_spmd

    logits = np.asarray(logits, dtype=np.float32)
    transitions = np.asarray(transitions, dtype=np.float32)
    lens_in = np.asarray(lens)

    prep = _host_prep(logits, transitions, lens_in, M=M_SLOTS)

    t0 = time.time()
    nc = _build_bass(prep["NSLOT"], prep["COLS"], prep["fwd_caps"],
                     prep["bwd_caps"], nf=prep["nf"], nb=prep["nb"])
    t1 = time.time()

    in_maps = [{"ef": prep["efs"][m], "eb": prep["ebs"][m],
                "wf": prep["wf"], "wb": prep["wb"]}
               for m in range(NCORES)]
    try:
        r = run_bass_kernel_spmd(nc, in_maps, core_ids=list(range(NCORES)))
    except Exception:
        time.sleep(10)
        r = run_bass_kernel_spmd(nc, in_maps, core_ids=list(range(NCORES)))
    t2 = time.time()

    capf = [r.results[m]["capf"] for m in range(NCORES)]
    capb = [r.results[m]["capb"] for m in range(NCORES)]
    out = _host_combine(prep, capf, capb)

    LAST.clear()
    LAST.update(build_s=t1 - t0, run_s=t2 - t1, results=r,
                exec_time_ns=r.exec_time_ns, nslot=prep["NSLOT"],
                cols=prep["COLS"])
    return out


if __name__ == "__main__":
    rng = np.random.default_rng(0)
    B, S = 512, 512
    logits = rng.standard_normal((B, S, L), dtype=np.float32)
    lens = rng.integers(1, S + 1, size=B).astype(np.int64)
    transitions = rng.standard_normal((L, L)).astype(np.float32)
    out = kernel(logits=logits, transitions=transitions, lens=lens)
    print("out[:8] =", out[:8])
    print("timings:", {k: LAST[k] for k in ("build_s", "run_s", "nslot")})



# revision 18
# speedup vs baseline: 484.6758x; 1.9817x over previous
import time
from contextlib import ExitStack

import numpy as np

BOS_IDX = 0
NCORES = 8
L = 128
M_SLOTS = 36
QT = 9

LAST = {}


def _bf16(x):
    import ml_dtypes
    return np.asarray(x).astype(ml_dtypes.bfloat16)


def _perron_c(transitions):
    W64 = np.exp(transitions.astype(np.float64))
    v = np.ones(L)
    for _ in range(200):
        v = W64 @ v
        v /= np.linalg.norm(v)
    lam1 = float(v @ W64 @ v) / float(v @ v)
    return float(np.log(lam1) + 0.5)


def _host_prep(logits, transitions, lens, M=M_SLOTS):
    B, S, Lc = logits.shape
    assert Lc == L
    lens = np.asarray(lens).astype(np.int64)

    c = _perron_c(transitions)
    Wp64 = np.exp(transitions.astype(np.float64) - c)
    wf = _bf16(np.ascontiguousarray(Wp64.T))
    wb = _bf16(np.ascontiguousarray(Wp64))
    z_pad64 = np.linalg.solve(Wp64.T, np.ones(L))

    elog = np.exp(logits.astype(np.float32))

    int_jobs = []
    tail_jobs = []
    seq_k = np.zeros(B, np.int64)
    for b in range(B):
        l = int(lens[b])
        k = max(1, -(-l // M))
        seq_k[b] = k
        cuts = [min(i * M, l) for i in range(k + 1)]
        for j in range(k - 1):
            int_jobs.append((b, cuts[j], cuts[j + 1]))
        tail_jobs.append((cuts[k] - cuts[k - 1], b, cuts[k - 1], cuts[k]))

    NI = -(-len(int_jobs) // NCORES)
    NI += NI % 2
    int_rank = [[None] * NI for _ in range(NCORES)]
    for i, job in enumerate(int_jobs):
        int_rank[i % NCORES][i // NCORES] = job

    tail_jobs.sort(key=lambda j: -j[0])
    NT = -(-len(tail_jobs) // NCORES)
    tail_rank = [[None] * NT for _ in range(NCORES)]
    for i, job in enumerate(tail_jobs):
        tail_rank[i % NCORES][i // NCORES] = job
    BL = np.zeros(NT, np.int64)
    for i in range(NT):
        mx = max((tail_rank[m][i][0] if tail_rank[m][i] else 1)
                 for m in range(NCORES))
        BL[i] = min(M, -(-mx // QT) * QT)

    NB = NI + NT
    NB += NB % 2
    NSLOT = M

    nb = np.zeros(NSLOT, np.int64)
    for s in range(NSLOT):
        at = np.nonzero(BL > s)[0]
        nb[s] = NI + ((at.max() + 1) if at.size else 0)

    bwd_caps = {}
    for i in range(NT):
        bwd_caps.setdefault(int(BL[i] - 1), []).append(NI + i)
    if NI > 0:
        bwd_caps.setdefault(NSLOT - 1, []).extend(range(NI))

    def runs(ks):
        out = []
        for kk in sorted(ks):
            if out and out[-1][1] == kk:
                out[-1] = (out[-1][0], kk + 1)
            else:
                out.append((kk, kk + 1))
        return out

    bwd_caps = {s: runs(v) for s, v in bwd_caps.items()}

    place = {}
    streams = []
    z_pad32 = z_pad64.astype(np.float32)
    for m in range(NCORES):
        eb = np.ones((NSLOT, L, NB), np.float32)
        for i in range(NI):
            job = int_rank[m][i]
            if job is None:
                continue
            b, t0, t1 = job
            eb[:, :, i] = elog[b, t0:t1, :][::-1]
            place[(b, 'i', t0)] = (m, i)
        for i in range(NT):
            job = tail_rank[m][i]
            if job is None:
                continue
            ln_, b, t0, t1 = job
            pad = int(BL[i]) - ln_
            if pad:
                eb[:pad, :, NI + i] = z_pad32[None, :]
            eb[pad:BL[i], :, NI + i] = elog[b, t0:t1, :][::-1]
            place[(b, 't', t0)] = (m, NI + i)
        streams.append(_bf16(np.ascontiguousarray(
            eb.transpose(1, 0, 2).reshape(L, NSLOT * NB))))

    return dict(c=c, wf=wf, wb=wb, streams=streams, NSLOT=NSLOT, NI=NI,
                NB=NB, nb=nb, bwd_caps=bwd_caps, place=place, lens=lens,
                seq_k=seq_k, M=M)


def _host_combine(prep, capf, capb):
    c = prep["c"]
    lens = prep["lens"]
    seq_k = prep["seq_k"]
    place = prep["place"]
    M = prep["M"]
    B = len(lens)
    ones = np.ones(L, np.float64)

    def vecA(b, t0):
        m, i = place[(b, 'i', t0)]
        return capf[m][:, i].astype(np.float64)

    def vecB(b, t0):
        m, i = place[(b, 'i', t0)]
        return capb[m][:, i].astype(np.float64)

    def vecH(b, t0):
        m, i = place[(b, 't', t0)]
        return capb[m][:, i].astype(np.float64)

    logZ = np.empty(B, np.float64)
    for b in range(B):
        l = int(lens[b])
        k = int(seq_k[b])
        cuts = [min(i * M, l) for i in range(k + 1)]
        H = vecH(b, cuts[k - 1])
        if k == 1:
            logZ[b] = np.log(H[BOS_IDX]) + c * l
            continue
        A_last = vecA(b, cuts[k - 2])
        lz = np.log(H @ A_last)
        for j in range(2, k):
            Bj = vecB(b, cuts[j - 1])
            Aprev = vecA(b, cuts[j - 2])
            lz += np.log(Bj @ Aprev) - np.log(Bj @ ones)
        B1 = vecB(b, cuts[0])
        lz += np.log(B1[BOS_IDX]) - np.log(B1 @ ones)
        logZ[b] = lz + c * l
    return logZ.astype(np.float32)


def _default_splits(NI, NB):
    return ((NI, 'D'),), ((NB, 'D'),)


def _build_bass(NSLOT, NI, NB, nb, bwd_caps, repeat=1, timing=False,
                noserial=False, caps=True, fsplit=None, bsplit=None):
    import concourse.bacc as bacc
    import concourse.mybir as mybir
    import concourse.tile as tile

    f32 = mybir.dt.float32
    bf16 = mybir.dt.bfloat16
    if fsplit is None or bsplit is None:
        fsplit, bsplit = _default_splits(NI, NB)
    assert sum(w for w, _ in fsplit) == NI
    assert sum(w for w, _ in bsplit) == NB
    nc = bacc.Bacc("TRN2", target_bir_lowering=False, debug=False,
                   num_devices=NCORES)

    stream_kind = "Internal" if timing else "ExternalInput"
    eb_d = nc.dram_tensor("eb", [L, NSLOT * NB], bf16, kind=stream_kind).ap()
    wf_d = nc.dram_tensor("wf", [L, L], bf16, kind="ExternalInput").ap()
    wb_d = nc.dram_tensor("wb", [L, L], bf16, kind="ExternalInput").ap()
    capf_d = nc.dram_tensor("capf", [L, NI], f32, kind="ExternalOutput").ap()
    capb_d = nc.dram_tensor("capb", [L, NB], f32, kind="ExternalOutput").ap()

    CH = max(4, (NSLOT + 3) // 4)
    nch = -(-NSLOT // CH)
    lo = [(i * CH, min(NSLOT, (i + 1) * CH)) for i in range(nch)]
    order = []
    a, bidx = 0, nch - 1
    while a <= bidx:
        if a != bidx:
            order.extend([lo[bidx], lo[a]])
        else:
            order.append(lo[a])
        a += 1
        bidx -= 1

    fchains = []
    off = 0
    for w, path in fsplit:
        fchains.append((off, off + w, path))
        off += w
    bchains = []
    off = 0
    for w, path in bsplit:
        bchains.append((off, off + w, path))
        off += w

    with tile.TileContext(nc) as tc, ExitStack() as ctx:
        cpool = ctx.enter_context(tc.tile_pool(name="const", bufs=1))
        spool = ctx.enter_context(tc.tile_pool(name="state", bufs=3))
        ypool = ctx.enter_context(tc.tile_pool(name="evac", bufs=2))
        strm = ctx.enter_context(tc.tile_pool(name="stream", bufs=2))
        psf = [ctx.enter_context(
            tc.tile_pool(name=f"psf{g}", bufs=1, space="PSUM"))
            for g in range(len(fchains))]
        psb = [ctx.enter_context(
            tc.tile_pool(name=f"psb{g}", bufs=1, space="PSUM"))
            for g in range(len(bchains))]

        wf_t = cpool.tile([L, L], bf16, tag="wf")
        nc.sync.dma_start(wf_t[:], wf_d[:])
        wb_t = cpool.tile([L, L], bf16, tag="wb")
        nc.sync.dma_start(wb_t[:], wb_d[:])

        capF = cpool.tile([L, NI], f32, tag="capF")
        nc.vector.memset(capF[:], 1.0)
        capB = cpool.tile([L, NB], f32, tag="capB")
        nc.vector.memset(capB[:], 1.0)

        serial = timing and not noserial
        loop = timing
        if serial:
            pcar = [cpool.tile([L, 8], bf16, tag=f"pcar{g}", name=f"pcar{g}")
                    for g in range(len(fchains))]
            hcar = [cpool.tile([L, 8], bf16, tag=f"hcar{g}", name=f"hcar{g}")
                    for g in range(len(bchains))]
            for t in pcar + hcar:
                nc.vector.memset(t[:], 1.0)

        def one_pass():
            S = strm.tile([L, NSLOT * NB], bf16, tag="S")
            for s0, s1 in order:
                nc.sync.dma_start(S[:, s0 * NB:s1 * NB],
                                  eb_d[:, s0 * NB:s1 * NB])

            def ef_ap(s, c0, c1):
                off = (NSLOT - 1 - s) * NB
                return S[:, off + c0:off + c1]

            def eb_ap(s, c0, c1):
                off = s * NB
                return S[:, off + c0:off + c1]

            ps = []
            for g, (c0, c1, _) in enumerate(fchains):
                p = spool.tile([L, c1 - c0], bf16, tag=f"p{g}")
                nc.vector.memset(p[:], 1.0)
                if serial:
                    nc.scalar.copy(p[:, :8], pcar[g][:])
                ps.append(p)

            vbs = [None] * len(bchains)
            if serial:
                for g, (c0, c1, _) in enumerate(bchains):
                    aw = max(0, min(c1, int(nb[0])) - c0)
                    if aw <= 0:
                        continue
                    vb = spool.tile([L, aw], bf16, tag=f"vb{g}")
                    nc.vector.tensor_copy(vb[:], eb_ap(0, c0, c0 + aw))
                    nc.vector.tensor_mul(vb[:, :8], hcar[g][:],
                                         eb_ap(0, c0, c0 + 8))
                    vbs[g] = vb

            qbs = [None] * len(bchains)
            for s in range(NSLOT):
                last = s == NSLOT - 1
                for g, (c0, c1, path) in enumerate(fchains):
                    w = c1 - c0
                    qf = psf[g].tile([L, w], f32, tag=f"qf{g}")
                    nc.tensor.matmul(qf[:], wf_t[:], ps[g][:])
                    if last:
                        nc.vector.tensor_mul(capF[:, c0:c1], qf[:],
                                             ef_ap(s, c0, c1))
                    elif path == 'Y':
                        qf_sb = ypool.tile([L, w], bf16, tag=f"qfs{g}")
                        nc.scalar.copy(qf_sb[:], qf[:])
                        ps[g] = spool.tile([L, w], bf16, tag=f"p{g}", name=f"p{g}")
                        nc.vector.tensor_mul(ps[g][:], qf_sb[:],
                                             ef_ap(s, c0, c1))
                    else:
                        ps[g] = spool.tile([L, w], bf16, tag=f"p{g}", name=f"p{g}")
                        nc.vector.tensor_mul(ps[g][:], qf[:],
                                             ef_ap(s, c0, c1))

                for g, (c0, c1, path) in enumerate(bchains):
                    aw = max(0, min(c1, int(nb[s])) - c0)
                    if aw <= 0:
                        continue
                    qb = psb[g].tile([L, aw], f32, tag=f"qb{g}")
                    nc.tensor.matmul(
                        qb[:], wb_t[:],
                        vbs[g][:, :aw] if vbs[g] is not None
                        else eb_ap(s, c0, c0 + aw))
                    qbs[g] = qb
                    if caps:
                        for lo_, hi_ in bwd_caps.get(s, []):
                            lo2, hi2 = max(lo_, c0), min(hi_, c1)
                            if lo2 < hi2:
                                nc.scalar.copy(capB[:, lo2:hi2],
                                               qb[:, lo2 - c0:hi2 - c0])
                    if last:
                        continue
                    nxt = max(0, min(c1, int(nb[s + 1])) - c0)
                    if nxt <= 0:
                        vbs[g] = None
                        continue
                    if path == 'Y':
                        hb_sb = ypool.tile([L, nxt], bf16, tag=f"hbs{g}")
                        nc.scalar.copy(hb_sb[:], qb[:, :nxt])
                        vbs[g] = spool.tile([L, nxt], bf16, tag=f"vb{g}", name=f"vb{g}")
                        nc.vector.tensor_mul(vbs[g][:], hb_sb[:],
                                             eb_ap(s + 1, c0, c0 + nxt))
                    else:
                        vbs[g] = spool.tile([L, nxt], bf16, tag=f"vb{g}", name=f"vb{g}")
                        nc.vector.tensor_mul(vbs[g][:], qb[:, :nxt],
                                             eb_ap(s + 1, c0, c0 + nxt))

            if serial:
                for g, (c0, c1, _) in enumerate(fchains):
                    nc.scalar.copy(pcar[g][:], capF[:, c0:c0 + 8])
                for g in range(len(bchains)):
                    if qbs[g] is not None:
                        nc.scalar.copy(hcar[g][:], qbs[g][:, :8])

        if loop:
            with tc.For_i(0, repeat):
                one_pass()
        else:
            assert repeat == 1
            one_pass()

        nc.sync.dma_start(capf_d[:], capF[:])
        nc.sync.dma_start(capb_d[:], capB[:])

    nc.compile()
    return nc


def kernel(logits, transitions, lens):
    from concourse.bass_utils import run_bass_kernel_spmd

    logits = np.asarray(logits, dtype=np.float32)
    transitions = np.asarray(transitions, dtype=np.float32)
    lens_in = np.asarray(lens)

    prep = _host_prep(logits, transitions, lens_in, M=M_SLOTS)

    t0 = time.time()
    nc = _build_bass(prep["NSLOT"], prep["NI"], prep["NB"], prep["nb"],
                     prep["bwd_caps"])
    t1 = time.time()

    in_maps = [{"eb": prep["streams"][m], "wf": prep["wf"], "wb": prep["wb"]}
               for m in range(NCORES)]
    try:
        r = run_bass_kernel_spmd(nc, in_maps, core_ids=list(range(NCORES)))
    except Exception:
        time.sleep(10)
        r = run_bass_kernel_spmd(nc, in_maps, core_ids=list(range(NCORES)))
    t2 = time.time()

    capf = [r.results[m]["capf"] for m in range(NCORES)]
    capb = [r.results[m]["capb"] for m in range(NCORES)]
    out = _host_combine(prep, capf, capb)

    LAST.clear()
    LAST.update(build_s=t1 - t0, run_s=t2 - t1, results=r,
                exec_time_ns=r.exec_time_ns, nslot=prep["NSLOT"],
                cols=prep["NB"])
    return out


if __name__ == "__main__":
    rng = np.random.default_rng(0)
    B, S = 512, 512
    logits = rng.standard_normal((B, S, L), dtype=np.float32)
    lens = rng.integers(1, S + 1, size=B).astype(np.int64)
    transitions = rng.standard_normal((L, L)).astype(np.float32)
    out = kernel(logits=logits, transitions=transitions, lens=lens)
    print("out[:8] =", out[:8])
    print("timings:", {k: LAST[k] for k in ("build_s", "run_s", "nslot")})
